# revision 16
# baseline (speedup 1.0000x reference)
"""Trainium2 Bass kernel for a FlowNet-style CorrelationLayer.

out[0, j*7+i, h, w] = sum_c x[0,c,h,w] * y[0,c,h+j-3, w+i-3]   (zero-padded y)

Shapes: x, y = [1, 128, 384, 512] fp32  ->  out = [1, 49, 384, 512] fp32.

Strategy (v2)
-------------
* Shard H (rows) across the 8 NeuronCores: core k computes output rows
  [48k, 48k+48).  The y halo (3 rows each side) is sliced on the host, so no
  inter-core communication is needed.
* y ships as fp8 e3m4 (half the bytes of fp16) and feeds the TensorEngine
  rhs DIRECTLY: the PE upconverts operands to ~fp22 internally, so a mixed
  fp16(x-weights) x fp8e3(y) matmul is exact in the quantized values.  The
  e3m4 quantization of one operand costs ~1.34e-2 relative error (measured),
  well under the 2e-2 gate.  Quantizing BOTH operands (1.9e-2) is too risky,
  so x stays fp16.
* Patches are 16 rows x 4 cols (M=64, two patches col-tiled per PSUM tile).
  The y halo patch is 22 x 10, N=220.  Narrow (PB=4) patches shrink the
  shipped v-extent to 10, cutting output amplification to 8*10/49 = 1.63
  (vs 2.29 at 8x8): out = 3.94 MB/core vs 5.96.
* Staging st[m, u, t, v] per patch-row pr; the output DMA ships, per a-PAIR
  of x rows, the shared u-window [2ap, 2ap+8) as one big contiguous run per
  partition (the BIR verifier only allows whole-partition dim0 strides, so
  the exact per-pixel windows are finished by a cheap host-side gather).
  The last patch-row ships in two t-halves so the final transfer is small.
* Input chunks are spread over four dispatch queues (sync/gpsimd/tensor/
  scalar engines) sized so each chunk lands just before the matmuls that
  need it; outputs queue behind inputs on sync/gpsimd (the stream is
  input-bound until ~40us anyway) and the final half-tile fans out across
  four engines to minimize dispatch-serialized tail.
* Per-core HBM traffic: 6.3 (x fp16) + 3.6 (y fp8) + 3.9 (out) = 13.8 MB
  vs 19.4 MB for the v1 kernel.
"""

import numpy as np
import ml_dtypes

import bass_rust
import concourse.bass as bass  # noqa: F401  (AP types pulled in transitively)
import concourse.tile as tile
from concourse import bacc, mybir
from concourse.instruction_name_ordered_set import InstructionNameOrderedSet
from concourse.bass_utils import run_bass_kernel_spmd

B, C, H, W = 1, 128, 384, 512
NCORES = 8
HB = H // NCORES          # 48 output rows per core
PA, PB = 16, 4            # x patch: 16 rows x 4 cols = 64 = M per matmul
HA, HV = PA + 6, PB + 6   # y halo patch: 22 x 10
NF = HA * HV              # 220 = N (matmul free size)
PR = HB // PA             # 3 patch-rows
PW = W // PB              # 128 patch-cols
NQ = PW // 2              # 64 col-tile pairs (two patches per 128 partitions)
STF = HA * NQ * HV        # 14080 staging elems per partition per tile
RUNF = 10 * NQ * HV       # 6400: run elems per (a-quad) full-tile DMA
STF2 = HA * (NQ // 2) * HV  # 7040 staging elems per partition, half tiles
RUNH = 10 * (NQ // 2) * HV  # 3200: run elems per (a-quad) half-tile DMA

F16 = mybir.dt.float16
F8 = mybir.dt.float8e3
E3M4 = ml_dtypes.float8_e3m4

_PROGRAM = None


def _build_program():
    nc = bacc.Bacc("TRN2", target_bir_lowering=False, debug=False)

    # x pre-tiled on the host to [C, patch, m] (m = a*4 + b, a-major) so each
    # patch's 64 weight columns are contiguous.
    xb = nc.declare_dram_parameter("xb", [C, PR * PW, PA * PB], F16, isOutput=False)
    yb = nc.declare_dram_parameter("yb", [C, HB + 6, W + 6], F8, isOutput=False)
    # coa[pr, ap, half, p, (u_rel, q, v)]: p = 8 partitions covering
    # a in {2ap, 2ap+1} x b, u_rel = u - 2ap in [0, 8).
    coa = nc.declare_dram_parameter("coa", [2, 4, 32, RUNF], F16, isOutput=True)
    # cob[kk, ap, half, p, (u_rel, tr, v)]: pr 2 shipped in two t-halves kk.
    cob = nc.declare_dram_parameter("cob", [2, 4, 32, RUNH], F16, isOutput=True)

    with tile.TileContext(nc) as tc:
        with (
            tc.tile_pool(name="xpool", bufs=1) as xpool,
            tc.tile_pool(name="ypool", bufs=1) as ypool,
            tc.tile_pool(name="psum", bufs=6, space="PSUM") as psum_pool,
            tc.tile_pool(name="st", bufs=1) as st_pool,
        ):
            X = xpool.tile([C, PR * PW, PA * PB], F16)
            Y = ypool.tile([C, HB + 6, W + 6], F8)
            # st[m, u, t, v]: u-major so an a-pair's u-window is one
            # contiguous run per partition; (t, v) innermost so evacuation
            # writes land contiguously per (u, s) step.
            # pr 0/1 stage whole; pr 2 stages in two t-halves so each half
            # ships as its own full-tile descriptor set (3-dim APs only).
            ST = [
                st_pool.tile([128, HA, NQ, HV], F16, name=f"st{k}") for k in range(2)
            ]
            ST2 = [
                st_pool.tile([128, HA, NQ // 2, HV], F16, name=f"st2{k}")
                for k in range(2)
            ]

            # Input loads, spread over four dispatch queues (one per engine)
            # and ordered so each queue's cumulative bytes arrive just before
            # the matmuls that need them.  y0 (22 rows) is the long pole for
            # the first matmul, so it is split three ways; x patches stream
            # in quarter/half-pr chunks.
            # The Tile scheduler reorders same-engine DMA dispatches, which
            # can push x chunks ahead of the y rows the first matmuls need
            # (each engine's HW queue drains FIFO).  Pin only each engine's
            # FIRST load (a y0 chunk) via order-only (nosync) deps on every
            # later load; the scheduler orders the rest by its own model.
            first_load = {}

            def _ordered(inst, eng):
                p = first_load.get(eng.engine)
                if p is None:
                    first_load[eng.engine] = inst
                else:
                    deps = InstructionNameOrderedSet()
                    deps.add(p.ins.name)
                    inst.ins.add_nosync_dependencies_from(deps)

            def ly(r0, r1, eng, c0=0, c1=W + 6):
                _ordered(
                    eng.dma_start(Y[:, r0:r1, c0:c1], yb[:, r0:r1, c0:c1]), eng
                )

            def lx(p0, p1, eng):
                _ordered(eng.dma_start(X[:, p0:p1, :], xb[:, p0:p1, :]), eng)

            # Only gpsimd/sync/scalar can issue DMAs.  scalar fires all its
            # loads early (before its first evacuation); sync/gpsimd carry
            # the rest in need order.  The first batch (y rows 0-22 split
            # three ways + the first 4 x patches) lands ~14us so the matmul
            # pipeline starts as early as possible.
            ly(0, 8, nc.sync, 0, 262)
            ly(8, 15, nc.gpsimd, 0, 262)
            ly(15, 22, nc.scalar, 0, 262)
            lx(0, 4, nc.scalar)
            ly(0, 8, nc.sync, 262, 518)
            ly(8, 15, nc.gpsimd, 262, 518)
            ly(15, 22, nc.scalar, 262, 518)
            lx(4, 16, nc.sync)
            lx(16, 32, nc.gpsimd)
            lx(32, 48, nc.scalar)
            lx(48, 64, nc.sync)
            lx(64, 96, nc.gpsimd)
            lx(96, 128, nc.scalar)
            ly(22, 30, nc.sync)       # y for pr 1
            ly(30, 38, nc.gpsimd)
            lx(128, 160, nc.scalar)
            lx(160, 192, nc.sync)
            lx(192, 224, nc.gpsimd)
            lx(224, 256, nc.scalar)
            ly(38, 46, nc.sync)       # y for pr 2
            ly(46, 54, nc.gpsimd)
            lx(256, 288, nc.sync)
            lx(288, 320, nc.gpsimd)
            lx(320, 352, nc.sync)
            lx(352, 384, nc.gpsimd)

            # Pixel m-order puts a-quad g's 32 pixels (both col-tile
            # halves) at partitions {g, g+4, ..., g+124}: one stride-4
            # 32-step descriptor per a-quad.  Each SDMA engine serves a
            # fixed interleaved 8-partition set, so a stride-4 descriptor
            # engages ALL 16 engines (8 contiguous partitions would engage
            # only 2), and there are just 4 x 410 KB descriptors per tile.
            def ship_full(pr, eng_of):
                st_t = ST[pr][:, :, :].tensor
                for g in range(4):
                    src = bass_rust.AP(
                        st_t,
                        g * STF + (4 * g) * (NQ * HV),
                        [[4 * STF, 32], [1, RUNF]],
                    )
                    eng_of(g).dma_start(coa[pr, g], src)

            def ship_half(kk, eng_of):
                st_t = ST2[kk][:, :, :].tensor
                for g in range(4):
                    src = bass_rust.AP(
                        st_t,
                        g * STF2 + (4 * g) * ((NQ // 2) * HV),
                        [[4 * STF2, 32], [1, RUNH]],
                    )
                    eng_of(g).dma_start(cob[kk, g], src)

            for pr in range(PR):
                for qq in range(0, NQ, 2):
                    if pr < 2:
                        st, toff = ST[pr], qq
                    else:
                        st, toff = ST2[qq >= NQ // 2], qq % (NQ // 2)
                    # Four 16x4 patches (two col-tiled pairs) share one PSUM
                    # bank: [128, 2, 220] fp32 = 1760 B of the 2 KB bank.
                    ps = psum_pool.tile([128, 2, NF], mybir.dt.float32)
                    for s in range(2):
                        q = qq + s
                        for half in range(2):
                            wp = 2 * q + half
                            lhsT = X[:, pr * PW + wp, :]
                            rhs = Y[
                                :, pr * PA : pr * PA + HA, wp * PB : wp * PB + HV
                            ]
                            nc.tensor.matmul(
                                ps[half * 64 : (half + 1) * 64, s, :NF],
                                lhsT,
                                rhs,
                                start=True,
                                stop=True,
                                tile_position=(0, 64 * half),
                            )
                    # Evacuate (fp32 -> fp16) into st[m, u, t, v]; src walked
                    # (s, u, v) so the PSUM read is one contiguous 440-elem
                    # run per partition.  Alternate DVE / ACT.
                    dst = st[:, :, toff : toff + 2, :].rearrange("p u s v -> p s u v")
                    src = ps[:, :, :NF].rearrange("p s (u v) -> p s u v", u=HA)
                    if (qq // 2) % 2 == 0:
                        nc.vector.tensor_copy(dst, src)
                    else:
                        nc.scalar.copy(dst, src)
                    if pr == 2 and qq == NQ // 2 - 2:
                        # First t-half of the last patch-row is complete:
                        # ship it while the second half computes.
                        ship_half(0, lambda g: nc.gpsimd)
                if pr < 2:
                    ship_full(pr, lambda g: nc.gpsimd if g >= 2 else nc.sync)
            # Tail: fan the final 16 descriptors across the three DMA-capable
            # engines.  scalar's DMA queue has been idle since its early
            # input loads, so its ring has free slots immediately.
            # scalar and gpsimd dispatch promptly at this point (sync's
            # HWDGE ring may still be draining), so the tail avoids sync.
            ship_half(1, lambda g: nc.scalar if g % 2 else nc.gpsimd)

    nc.compile()
    return nc


def _program():
    global _PROGRAM
    if _PROGRAM is None:
        _PROGRAM = _build_program()
    return _PROGRAM


def _make_in_maps(x: np.ndarray, y: np.ndarray):
    x0 = np.asarray(x[0]).astype(np.float16)
    # [C, H, W] -> [C, H/PA, q4, ar4, PW, PB] -> [C, H/PA, PW, ar4, PB, q4]
    # (pixel (a=4*q4+ar4, b) sits at lhsT column (ar4*4 + b)*4 + q4, putting
    # each a-quad's pixels at stride-4 partitions for the output DMA).
    xt = x0.reshape(C, H // PA, PA // 4, 4, PW, PB).transpose(0, 1, 4, 3, 5, 2)
    xt = np.ascontiguousarray(xt.reshape(C, H // PA * PW, PA * PB))
    yp = np.zeros((C, H + 6, W + 6), E3M4)
    yp[:, 3 : 3 + H, 3 : 3 + W] = np.asarray(y[0]).astype(E3M4)
    in_maps = []
    for k in range(NCORES):
        in_maps.append(
            {
                "xb": np.ascontiguousarray(xt[:, k * PR * PW : (k + 1) * PR * PW, :]),
                "yb": np.ascontiguousarray(yp[:, k * HB : k * HB + HB + 6, :]),
            }
        )
    return in_maps


_GATHER_IDX = None


def _gather_indices():
    global _GATHER_IDX
    if _GATHER_IDX is None:
        j = np.arange(7)[None, :]
        i = np.arange(7)[None, :]
        ar = np.arange(4)[:, None]
        b = np.arange(PB)[:, None]
        _GATHER_IDX = (
            np.ascontiguousarray((ar + j).reshape(1, 1, 1, 4, 1, 7, 1, 1)),
            np.ascontiguousarray((b + i).reshape(1, 1, 1, 1, PB, 1, 1, 7)),
        )
    return _GATHER_IDX


def _gather_core(coa_k: np.ndarray, cob_k: np.ndarray) -> np.ndarray:
    """Device outputs -> [49, HB, W] band of the output."""
    iu, iv = _gather_indices()
    # [pr, q4, half, ar4, b, urel, q, v]: urel = ar4 + j, then v = b + i
    a = coa_k.reshape(2, 4, 2, 4, PB, 10, NQ, HV)
    g = np.take_along_axis(a, iu, axis=5)
    g = np.take_along_axis(g, iv, axis=7)
    # -> [j, i, pr, q4, ar4, q, half, b] = rows 0..31
    top = g.transpose(5, 7, 0, 1, 3, 6, 2, 4).reshape(49, 32, W)
    # [kk, q4, half, ar4, b, urel, tr, v]
    bb = cob_k.reshape(2, 4, 2, 4, PB, 10, NQ // 2, HV)
    g = np.take_along_axis(bb, iu, axis=5)
    g = np.take_along_axis(g, iv, axis=7)
    # -> [j, i, q4, ar4, kk, tr, half, b] = rows 32..47
    bot = g.transpose(5, 7, 1, 3, 0, 6, 2, 4).reshape(49, 16, W)
    return np.concatenate([top, bot], axis=1)


def _run(in_maps, trace=False, **kw):
    return run_bass_kernel_spmd(
        _program(), in_maps, core_ids=list(range(NCORES)), trace=trace, **kw
    )


def kernel(x: np.ndarray, y: np.ndarray) -> np.ndarray:
    x = np.asarray(x)
    y = np.asarray(y)
    res = _run(_make_in_maps(x, y)).results
    out = np.empty((1, 49, H, W), np.float32)
    for k in range(NCORES):
        out[0, :, k * HB : (k + 1) * HB, :] = _gather_core(
            np.asarray(res[k]["coa"]), np.asarray(res[k]["cob"])
        ).astype(np.float32)
    return out


# revision 17
# speedup vs baseline: 1.3635x; 1.3635x over previous
"""Trainium2 Bass kernel for a FlowNet-style CorrelationLayer.

out[0, j*7+i, h, w] = sum_c x[0,c,h,w] * y[0,c,h+j-3, w+i-3]   (zero-padded y)

Shapes: x, y = [1, 128, 384, 512] fp32  ->  out = [1, 49, 384, 512] fp32.

Strategy (v2)
-------------
* Shard H (rows) across the 8 NeuronCores: core k computes output rows
  [48k, 48k+48).  The y halo (3 rows each side) is sliced on the host, so no
  inter-core communication is needed.
* y ships as fp8 e3m4 (half the bytes of fp16) and feeds the TensorEngine
  rhs DIRECTLY: the PE upconverts operands to ~fp22 internally, so a mixed
  fp16(x-weights) x fp8e3(y) matmul is exact in the quantized values.  The
  e3m4 quantization of one operand costs ~1.34e-2 relative error (measured),
  well under the 2e-2 gate.  Quantizing BOTH operands (1.9e-2) is too risky,
  so x stays fp16.
* Patches are 16 rows x 4 cols (M=64, two patches col-tiled per PSUM tile).
  The y halo patch is 22 x 10, N=220.  Narrow (PB=4) patches shrink the
  shipped v-extent to 10, cutting output amplification to 8*10/49 = 1.63
  (vs 2.29 at 8x8): out = 3.94 MB/core vs 5.96.
* Staging st[m, u, t, v] per patch-row pr; the output DMA ships, per a-PAIR
  of x rows, the shared u-window [2ap, 2ap+8) as one big contiguous run per
  partition (the BIR verifier only allows whole-partition dim0 strides, so
  the exact per-pixel windows are finished by a cheap host-side gather).
  The last patch-row ships in two t-halves so the final transfer is small.
* Input chunks are spread over four dispatch queues (sync/gpsimd/tensor/
  scalar engines) sized so each chunk lands just before the matmuls that
  need it; outputs queue behind inputs on sync/gpsimd (the stream is
  input-bound until ~40us anyway) and the final half-tile fans out across
  four engines to minimize dispatch-serialized tail.
* Per-core HBM traffic: 6.3 (x fp16) + 3.6 (y fp8) + 3.9 (out) = 13.8 MB
  vs 19.4 MB for the v1 kernel.
"""

import numpy as np
import ml_dtypes

import bass_rust
import concourse.bass as bass  # noqa: F401  (AP types pulled in transitively)
import concourse.tile as tile
from concourse import bacc, mybir
from concourse.instruction_name_ordered_set import InstructionNameOrderedSet
from concourse.bass_utils import run_bass_kernel_spmd

B, C, H, W = 1, 128, 384, 512
NCORES = 8
HB = H // NCORES          # 48 output rows per core
PA, PB = 16, 4            # x patch: 16 rows x 4 cols = 64 = M per matmul
HA, HV = PA + 6, PB + 6   # y halo patch: 22 x 10
NF = HA * HV              # 220 = N (matmul free size)
PR = HB // PA             # 3 patch-rows
PW = W // PB              # 128 patch-cols
NQ = PW // 2              # 64 col-tile pairs (two patches per 128 partitions)
STF = HA * NQ * HV        # 14080 staging elems per partition per tile
RUNF = 10 * NQ * HV       # 6400: run elems per (a-quad) full-tile DMA
STF2 = HA * (NQ // 2) * HV  # 7040 staging elems per partition, half tiles
RUNH = 10 * (NQ // 2) * HV  # 3200: run elems per (a-quad) half-tile DMA

F16 = mybir.dt.float16
F8 = mybir.dt.float8e3
E3M4 = ml_dtypes.float8_e3m4

_PROGRAM = None


def _build_program():
    nc = bacc.Bacc("TRN2", target_bir_lowering=False, debug=False)

    # x pre-tiled on the host to [C, patch, m] (m = a*4 + b, a-major) so each
    # patch's 64 weight columns are contiguous.
    xb = nc.declare_dram_parameter("xb", [C, PR * PW, PA * PB], F16, isOutput=False)
    yb = nc.declare_dram_parameter("yb", [C, HB + 6, W + 6], F8, isOutput=False)
    # coa[pr, ap, half, p, (u_rel, q, v)]: p = 8 partitions covering
    # a in {2ap, 2ap+1} x b, u_rel = u - 2ap in [0, 8).
    coa = nc.declare_dram_parameter("coa", [2, 4, 32, RUNF], F16, isOutput=True)
    # cob[kk, ap, half, p, (u_rel, tr, v)]: pr 2 shipped in two t-halves kk.
    cob = nc.declare_dram_parameter("cob", [2, 4, 32, RUNH], F16, isOutput=True)

    with tile.TileContext(nc) as tc:
        with (
            tc.tile_pool(name="xpool", bufs=1) as xpool,
            tc.tile_pool(name="ypool", bufs=1) as ypool,
            tc.tile_pool(name="psum", bufs=6, space="PSUM") as psum_pool,
            tc.tile_pool(name="st", bufs=1) as st_pool,
        ):
            X = xpool.tile([C, PR * PW, PA * PB], F16)
            Y = ypool.tile([C, HB + 6, W + 6], F8)
            # st[m, u, t, v]: u-major so an a-pair's u-window is one
            # contiguous run per partition; (t, v) innermost so evacuation
            # writes land contiguously per (u, s) step.
            # pr 0/1 stage whole; pr 2 stages in two t-halves so each half
            # ships as its own full-tile descriptor set (3-dim APs only).
            ST = [
                st_pool.tile([128, HA, NQ, HV], F16, name=f"st{k}") for k in range(2)
            ]
            ST2 = [
                st_pool.tile([128, HA, NQ // 2, HV], F16, name=f"st2{k}")
                for k in range(2)
            ]

            # Input loads, spread over four dispatch queues (one per engine)
            # and ordered so each queue's cumulative bytes arrive just before
            # the matmuls that need them.  y0 (22 rows) is the long pole for
            # the first matmul, so it is split three ways; x patches stream
            # in quarter/half-pr chunks.
            # The Tile scheduler reorders same-engine DMA dispatches, which
            # can push x chunks ahead of the y rows the first matmuls need
            # (each engine's HW queue drains FIFO).  Pin only each engine's
            # FIRST load (a y0 chunk) via order-only (nosync) deps on every
            # later load; the scheduler orders the rest by its own model.
            first_load = {}

            def _ordered(inst, eng):
                p = first_load.get(eng.engine)
                if p is None:
                    first_load[eng.engine] = inst
                else:
                    deps = InstructionNameOrderedSet()
                    deps.add(p.ins.name)
                    inst.ins.add_nosync_dependencies_from(deps)

            def ly(r0, r1, eng, c0=0, c1=W + 6):
                _ordered(
                    eng.dma_start(Y[:, r0:r1, c0:c1], yb[:, r0:r1, c0:c1]), eng
                )

            def lx(p0, p1, eng):
                _ordered(eng.dma_start(X[:, p0:p1, :], xb[:, p0:p1, :]), eng)

            # Only gpsimd/sync/scalar can issue DMAs.  scalar fires all its
            # loads early (before its first evacuation); sync/gpsimd carry
            # the rest in need order.  The first batch (y rows 0-22 split
            # three ways + the first 4 x patches) lands ~14us so the matmul
            # pipeline starts as early as possible.
            ly(0, 8, nc.sync)
            ly(8, 15, nc.gpsimd)
            ly(15, 22, nc.scalar)
            lx(0, 4, nc.scalar)
            lx(4, 16, nc.sync)
            lx(16, 32, nc.gpsimd)
            lx(32, 48, nc.scalar)
            lx(48, 64, nc.sync)
            lx(64, 96, nc.gpsimd)
            lx(96, 128, nc.scalar)
            ly(22, 30, nc.sync)       # y for pr 1
            ly(30, 38, nc.gpsimd)
            lx(128, 160, nc.scalar)
            lx(160, 192, nc.sync)
            lx(192, 224, nc.gpsimd)
            lx(224, 256, nc.scalar)
            ly(38, 46, nc.sync)       # y for pr 2
            ly(46, 54, nc.gpsimd)
            lx(256, 288, nc.sync)
            lx(288, 320, nc.gpsimd)
            lx(320, 352, nc.sync)
            lx(352, 384, nc.gpsimd)

            # Pixel m-order puts a-quad g's 32 pixels (both col-tile
            # halves) at partitions {g, g+4, ..., g+124}: one stride-4
            # 32-step descriptor per a-quad.  Each SDMA engine serves a
            # fixed interleaved 8-partition set, so a stride-4 descriptor
            # engages ALL 16 engines (8 contiguous partitions would engage
            # only 2), and there are just 4 x 410 KB descriptors per tile.
            def ship_full(pr, eng_of):
                st_t = ST[pr][:, :, :].tensor
                for g in range(4):
                    src = bass_rust.AP(
                        st_t,
                        g * STF + (4 * g) * (NQ * HV),
                        [[4 * STF, 32], [1, RUNF]],
                    )
                    eng_of(g).dma_start(coa[pr, g], src)

            def ship_half(kk, eng_of):
                st_t = ST2[kk][:, :, :].tensor
                for g in range(4):
                    src = bass_rust.AP(
                        st_t,
                        g * STF2 + (4 * g) * ((NQ // 2) * HV),
                        [[4 * STF2, 32], [1, RUNH]],
                    )
                    eng_of(g).dma_start(cob[kk, g], src)

            for pr in range(PR):
                for qq in range(0, NQ, 2):
                    if pr < 2:
                        st, toff = ST[pr], qq
                    else:
                        st, toff = ST2[qq >= NQ // 2], qq % (NQ // 2)
                    # Four 16x4 patches (two col-tiled pairs) share one PSUM
                    # bank: [128, 2, 220] fp32 = 1760 B of the 2 KB bank.
                    ps = psum_pool.tile([128, 2, NF], mybir.dt.float32)
                    for s in range(2):
                        q = qq + s
                        for half in range(2):
                            wp = 2 * q + half
                            lhsT = X[:, pr * PW + wp, :]
                            rhs = Y[
                                :, pr * PA : pr * PA + HA, wp * PB : wp * PB + HV
                            ]
                            nc.tensor.matmul(
                                ps[half * 64 : (half + 1) * 64, s, :NF],
                                lhsT,
                                rhs,
                                start=True,
                                stop=True,
                                tile_position=(0, 64 * half),
                            )
                    # Evacuate (fp32 -> fp16) into st[m, u, t, v]; src walked
                    # (s, u, v) so the PSUM read is one contiguous 440-elem
                    # run per partition.  Alternate DVE / ACT.
                    dst = st[:, :, toff : toff + 2, :].rearrange("p u s v -> p s u v")
                    src = ps[:, :, :NF].rearrange("p s (u v) -> p s u v", u=HA)
                    if (qq // 2) % 2 == 0:
                        nc.vector.tensor_copy(dst, src)
                    else:
                        nc.scalar.copy(dst, src)
                    if pr == 2 and qq == NQ // 2 - 2:
                        # First t-half of the last patch-row is complete:
                        # ship it while the second half computes.
                        ship_half(0, lambda g: nc.gpsimd)
                if pr < 2:
                    ship_full(pr, lambda g: nc.gpsimd if g >= 2 else nc.sync)
            # Tail: fan the final 16 descriptors across the three DMA-capable
            # engines.  scalar's DMA queue has been idle since its early
            # input loads, so its ring has free slots immediately.
            # scalar and gpsimd dispatch promptly at this point (sync's
            # HWDGE ring may still be draining), so the tail avoids sync.
            ship_half(1, lambda g: nc.scalar if g % 2 else nc.gpsimd)

    nc.compile()
    return nc


def _program():
    global _PROGRAM
    if _PROGRAM is None:
        _PROGRAM = _build_program()
    return _PROGRAM


def _make_in_maps(x: np.ndarray, y: np.ndarray):
    x0 = np.asarray(x[0]).astype(np.float16)
    # [C, H, W] -> [C, H/PA, q4, ar4, PW, PB] -> [C, H/PA, PW, ar4, PB, q4]
    # (pixel (a=4*q4+ar4, b) sits at lhsT column (ar4*4 + b)*4 + q4, putting
    # each a-quad's pixels at stride-4 partitions for the output DMA).
    xt = x0.reshape(C, H // PA, PA // 4, 4, PW, PB).transpose(0, 1, 4, 3, 5, 2)
    xt = np.ascontiguousarray(xt.reshape(C, H // PA * PW, PA * PB))
    yp = np.zeros((C, H + 6, W + 6), E3M4)
    yp[:, 3 : 3 + H, 3 : 3 + W] = np.asarray(y[0]).astype(E3M4)
    in_maps = []
    for k in range(NCORES):
        in_maps.append(
            {
                "xb": np.ascontiguousarray(xt[:, k * PR * PW : (k + 1) * PR * PW, :]),
                "yb": np.ascontiguousarray(yp[:, k * HB : k * HB + HB + 6, :]),
            }
        )
    return in_maps


_GATHER_IDX = None


def _gather_indices():
    global _GATHER_IDX
    if _GATHER_IDX is None:
        j = np.arange(7)[None, :]
        i = np.arange(7)[None, :]
        ar = np.arange(4)[:, None]
        b = np.arange(PB)[:, None]
        _GATHER_IDX = (
            np.ascontiguousarray((ar + j).reshape(1, 1, 1, 4, 1, 7, 1, 1)),
            np.ascontiguousarray((b + i).reshape(1, 1, 1, 1, PB, 1, 1, 7)),
        )
    return _GATHER_IDX


def _gather_core(coa_k: np.ndarray, cob_k: np.ndarray) -> np.ndarray:
    """Device outputs -> [49, HB, W] band of the output."""
    iu, iv = _gather_indices()
    # [pr, q4, half, ar4, b, urel, q, v]: urel = ar4 + j, then v = b + i
    a = coa_k.reshape(2, 4, 2, 4, PB, 10, NQ, HV)
    g = np.take_along_axis(a, iu, axis=5)
    g = np.take_along_axis(g, iv, axis=7)
    # -> [j, i, pr, q4, ar4, q, half, b] = rows 0..31
    top = g.transpose(5, 7, 0, 1, 3, 6, 2, 4).reshape(49, 32, W)
    # [kk, q4, half, ar4, b, urel, tr, v]
    bb = cob_k.reshape(2, 4, 2, 4, PB, 10, NQ // 2, HV)
    g = np.take_along_axis(bb, iu, axis=5)
    g = np.take_along_axis(g, iv, axis=7)
    # -> [j, i, q4, ar4, kk, tr, half, b] = rows 32..47
    bot = g.transpose(5, 7, 1, 3, 0, 6, 2, 4).reshape(49, 16, W)
    return np.concatenate([top, bot], axis=1)


def _run(in_maps, trace=False, **kw):
    return run_bass_kernel_spmd(
        _program(), in_maps, core_ids=list(range(NCORES)), trace=trace, **kw
    )


def kernel(x: np.ndarray, y: np.ndarray) -> np.ndarray:
    x = np.asarray(x)
    y = np.asarray(y)
    res = _run(_make_in_maps(x, y)).results
    out = np.empty((1, 49, H, W), np.float32)
    for k in range(NCORES):
        out[0, :, k * HB : (k + 1) * HB, :] = _gather_core(
            np.asarray(res[k]["coa"]), np.asarray(res[k]["cob"])
        ).astype(np.float32)
    return out
